# revision 3
# baseline (speedup 1.0000x reference)
"""AUAvULoss kernel for Trainium2, data-parallel over N across 8 NeuronCores.

Two SPMD launches:
  K1: streams probs/y/weights (12 MB/core), computes per-sample entropy (unc),
      confidence, correctness, the quadrant weight arrays A=conf*corr,
      P=A*tanh(unc), B=(1-conf)*(1-corr), Q=B*tanh(unc), plus partial
      CE / focal sums and per-core min/max of unc.
  host: all-reduce min/max -> 21 thresholds.
  K2: re-reads unc/A/P/B/Q (2.5 MB/core SBUF-resident) and computes the 84
      masked sums  S_le{A,P,B,Q}[t] = sum_i 1[unc_i <= th_t] * w_i  with fused
      scalar_tensor_tensor(accum_out) ops split across DVE and GpSimd.
  host: prefix algebra -> n_ac/n_au/n_ic/n_iu -> AvU AUC -> losses.
"""

import os
import sys
from contextlib import ExitStack

import numpy as np

for _p in ("/opt/trn_rl_repo",):
    if _p not in sys.path:
        sys.path.insert(0, _p)

import concourse.bacc as bacc
import concourse.bass as bass
import concourse.mybir as mybir
import concourse.tile as tile
from concourse.bass_utils import run_bass_kernel_spmd

f32 = mybir.dt.float32
AF = mybir.ActivationFunctionType
OP = mybir.AluOpType
AX = mybir.AxisListType

NCORES = 8
N, C = 1_000_000, 8
R = N // NCORES          # 125_000 rows per core
P = 125                  # SBUF partitions used
W = R // P               # 1000 rows per partition
NCH = 8                  # K1 chunks
CW = W // NCH            # 125 rows/partition per chunk
CE_W = CW * C            # 1000 elements/partition per chunk
NTH = 21
DVE_TH = 21              # all thresholds on DVE (Pool can't STT+accum)

EPS = 1e-10
BETA = 1.0


def build_k1(label_col):
    """label_col: class index in [0,8) that predictions are compared to,
    or None (labels scalar out of range -> nothing is 'correct')."""
    nc = bacc.Bacc("TRN2", target_bir_lowering=False, debug=False,
                   enable_asserts=False, num_devices=NCORES)
    pr_d = nc.dram_tensor("probs", [R, C], f32, kind="ExternalInput").ap()
    y_d = nc.dram_tensor("y", [R, C], f32, kind="ExternalInput").ap()
    w_d = nc.dram_tensor("w", [R, C], f32, kind="ExternalInput").ap()

    unc_d = nc.dram_tensor("unc", [P, W], f32, kind="ExternalOutput").ap()
    A_d = nc.dram_tensor("Aw", [P, W], f32, kind="ExternalOutput").ap()
    P_d = nc.dram_tensor("Pw", [P, W], f32, kind="ExternalOutput").ap()
    B_d = nc.dram_tensor("Bw", [P, W], f32, kind="ExternalOutput").ap()
    Q_d = nc.dram_tensor("Qw", [P, W], f32, kind="ExternalOutput").ap()
    ce_d = nc.dram_tensor("ce", [P, NCH], f32, kind="ExternalOutput").ap()
    fo_d = nc.dram_tensor("fo", [P, NCH], f32, kind="ExternalOutput").ap()
    ex_d = nc.dram_tensor("ex", [P, 4], f32, kind="ExternalOutput").ap()

    pr_r = pr_d.rearrange("(p w) c -> p (w c)", p=P)   # [125, 8000]
    y_r = y_d.rearrange("(p w) c -> p (w c)", p=P)
    w_r = w_d.rearrange("(p w) c -> p (w c)", p=P)

    with tile.TileContext(nc) as tc, ExitStack() as ctx:
        io = ctx.enter_context(tc.tile_pool(name="io", bufs=3))
        sc = ctx.enter_context(tc.tile_pool(name="sc", bufs=2))
        ps = ctx.enter_context(tc.tile_pool(name="ps", bufs=1))

        unc_t = ps.tile([P, W], f32, tag="unc")
        conf_t = ps.tile([P, W], f32, tag="conf")
        corr_t = ps.tile([P, W], f32, tag="corr")
        tanh_t = ps.tile([P, W], f32, tag="tanh")
        A_t = ps.tile([P, W], f32, tag="A")
        P_t = ps.tile([P, W], f32, tag="Pq")
        B_t = ps.tile([P, W], f32, tag="B")
        Q_t = ps.tile([P, W], f32, tag="Qq")
        ce_acc = ps.tile([P, NCH], f32, tag="ceacc")
        fo_acc = ps.tile([P, NCH], f32, tag="foacc")
        ex_t = ps.tile([P, 4], f32, tag="ex")
        ones_t = ps.tile([P, 1], f32, tag="ones")
        nc.vector.memset(ones_t[:], 1.0)

        for k in range(NCH):
            sl = bass.ts(k, CE_W)
            pr = io.tile([P, CE_W], f32, tag="pr")
            nc.sync.dma_start(pr[:], pr_r[:, sl])
            yy = io.tile([P, CE_W], f32, tag="yy")
            nc.sync.dma_start(yy[:], y_r[:, sl])
            ww = io.tile([P, CE_W], f32, tag="ww")
            nc.sync.dma_start(ww[:], w_r[:, sl])

            lg = sc.tile([P, CE_W], f32, tag="lg")
            nc.scalar.activation(lg[:], pr[:], AF.Ln)

            pl = sc.tile([P, CE_W], f32, tag="pl")
            nc.gpsimd.tensor_tensor(pl[:], pr[:], lg[:], op=OP.mult)

            pr3 = pr[:].rearrange("p (a c) -> p a c", c=C)
            pl3 = pl[:].rearrange("p (a c) -> p a c", c=C)
            ksl = bass.ts(k, CW)
            nc.vector.tensor_reduce(unc_t[:, ksl], pl3, axis=AX.X,
                                    op=OP.add, negate=True)
            nc.vector.tensor_reduce(conf_t[:, ksl], pr3, axis=AX.X, op=OP.max)

            t1 = sc.tile([P, CE_W], f32, tag="t1")
            nc.vector.scalar_tensor_tensor(
                out=t1[:], in0=yy[:], scalar=ones_t[:, 0:1], in1=lg[:],
                op0=OP.mult, op1=OP.mult, accum_out=ce_acc[:, k:k + 1])
            junk = sc.tile([P, CE_W], f32, tag="junk")
            nc.vector.scalar_tensor_tensor(
                out=junk[:], in0=t1[:], scalar=ones_t[:, 0:1], in1=ww[:],
                op0=OP.mult, op1=OP.mult, accum_out=fo_acc[:, k:k + 1])

            if label_col is not None:
                prL = pr3[:, :, label_col:label_col + 1]
                prL = prL.rearrange("p a c -> p (a c)")
                nc.vector.tensor_tensor(corr_t[:, ksl], prL,
                                        conf_t[:, ksl], op=OP.is_ge)
            else:
                nc.vector.memset(corr_t[:, ksl], 0.0)

        nc.scalar.activation(tanh_t[:], unc_t[:], AF.Tanh)
        nc.vector.tensor_tensor(A_t[:], conf_t[:], corr_t[:], op=OP.mult)
        nc.vector.tensor_tensor(P_t[:], A_t[:], tanh_t[:], op=OP.mult)
        s1 = sc.tile([P, W], f32, tag="s1")
        nc.vector.tensor_scalar_add(s1[:], conf_t[:], -1.0)
        s2 = sc.tile([P, W], f32, tag="s2")
        nc.vector.tensor_scalar_add(s2[:], corr_t[:], -1.0)
        nc.vector.tensor_tensor(B_t[:], s1[:], s2[:], op=OP.mult)
        nc.vector.tensor_tensor(Q_t[:], B_t[:], tanh_t[:], op=OP.mult)

        nc.vector.tensor_reduce(ex_t[:, 0:1], unc_t[:], axis=AX.X, op=OP.min)
        nc.vector.tensor_reduce(ex_t[:, 1:2], unc_t[:], axis=AX.X, op=OP.max)
        nc.vector.tensor_reduce(ex_t[:, 2:3], P_t[:], axis=AX.X, op=OP.add)
        nc.vector.tensor_reduce(ex_t[:, 3:4], Q_t[:], axis=AX.X, op=OP.add)

        nc.sync.dma_start(unc_d[:, :], unc_t[:])
        nc.sync.dma_start(A_d[:, :], A_t[:])
        nc.sync.dma_start(P_d[:, :], P_t[:])
        nc.sync.dma_start(B_d[:, :], B_t[:])
        nc.sync.dma_start(Q_d[:, :], Q_t[:])
        nc.sync.dma_start(ce_d[:, :], ce_acc[:])
        nc.sync.dma_start(fo_d[:, :], fo_acc[:])
        nc.sync.dma_start(ex_d[:, :], ex_t[:])

    nc.compile()
    return nc


def build_k2():
    nc = bacc.Bacc("TRN2", target_bir_lowering=False, debug=False,
                   enable_asserts=False, num_devices=NCORES)
    unc_d = nc.dram_tensor("unc", [P, W], f32, kind="ExternalInput").ap()
    A_d = nc.dram_tensor("Aw", [P, W], f32, kind="ExternalInput").ap()
    P_d = nc.dram_tensor("Pw", [P, W], f32, kind="ExternalInput").ap()
    B_d = nc.dram_tensor("Bw", [P, W], f32, kind="ExternalInput").ap()
    Q_d = nc.dram_tensor("Qw", [P, W], f32, kind="ExternalInput").ap()
    th_d = nc.dram_tensor("th", [P, NTH], f32, kind="ExternalInput").ap()
    av_d = nc.dram_tensor("accv", [P, 4 * DVE_TH], f32,
                          kind="ExternalOutput").ap()


    with tile.TileContext(nc) as tc, ExitStack() as ctx:
        ps = ctx.enter_context(tc.tile_pool(name="ps", bufs=1))
        sj = ctx.enter_context(tc.tile_pool(name="sj", bufs=2))

        unc_t = ps.tile([P, W], f32, tag="unc")
        nc.sync.dma_start(unc_t[:], unc_d[:, :])
        arrs = []
        for name, d in (("A", A_d), ("P", P_d), ("B", B_d), ("Q", Q_d)):
            t = ps.tile([P, W], f32, tag=name)
            nc.sync.dma_start(t[:], d[:, :])
            arrs.append(t)
        th_t = ps.tile([P, NTH], f32, tag="th")
        nc.sync.dma_start(th_t[:], th_d[:, :])
        acc_v = ps.tile([P, 4 * DVE_TH], f32, tag="accv")


        for t in range(NTH):
            for q in range(4):
                junk = sj.tile([P, W], f32, tag="jv")
                nc.vector.scalar_tensor_tensor(
                    out=junk[:], in0=unc_t[:], scalar=th_t[:, t:t + 1],
                    in1=arrs[q][:], op0=OP.is_le, op1=OP.mult,
                    accum_out=acc_v[:, 4 * t + q:4 * t + q + 1])

        nc.sync.dma_start(av_d[:, :], acc_v[:])

    nc.compile()
    return nc


_cache = {}


def _get_k1(label_col):
    key = ("k1", label_col)
    if key not in _cache:
        _cache[key] = build_k1(label_col)
    return _cache[key]


def _get_k2():
    if "k2" not in _cache:
        _cache["k2"] = build_k2()
    return _cache["k2"]


def _run(nc, in_maps, **kw):
    res = run_bass_kernel_spmd(nc, in_maps, core_ids=list(range(NCORES)), **kw)
    return res


def kernel(probs, y, weights, _results=None, _trace=False):
    probs = np.ascontiguousarray(probs, dtype=np.float32)
    y = np.ascontiguousarray(y, dtype=np.float32)
    weights = np.ascontiguousarray(weights, dtype=np.float32)

    flat_label = int(np.argmax(y))
    label_col = flat_label if flat_label < C else None

    nc1 = _get_k1(label_col)
    in1 = [{"probs": probs[i * R:(i + 1) * R],
            "y": y[i * R:(i + 1) * R],
            "w": weights[i * R:(i + 1) * R]} for i in range(NCORES)]
    tr1 = {"trace": True, "tmpdir": "/tmp/trace_k1"} if _trace else {}
    if _trace:
        import os as _os, shutil as _sh
        for d in ("/tmp/trace_k1", "/tmp/trace_k2"):
            _sh.rmtree(d, ignore_errors=True)
            _os.makedirs(d, exist_ok=True)
    r1 = _run(nc1, in1, **tr1)
    outs1 = r1.results

    ce_sum = sum(float(o["ce"].sum(dtype=np.float64)) for o in outs1)
    fo_sum = sum(float(o["fo"].sum(dtype=np.float64)) for o in outs1)
    CE_loss = -ce_sum / N
    focal_loss = -fo_sum / N
    umin = min(float(o["ex"][:, 0].min()) for o in outs1)
    umax = max(float(o["ex"][:, 1].max()) for o in outs1)
    SP = sum(float(o["ex"][:, 2].sum(dtype=np.float64)) for o in outs1)
    SQ = sum(float(o["ex"][:, 3].sum(dtype=np.float64)) for o in outs1)

    th01 = np.linspace(0.0, 1.0, NTH).astype(np.float32)
    unc_th = (np.float32(umin) + th01 *
              (np.float32(umax) - np.float32(umin))).astype(np.float32)
    th_b = np.broadcast_to(unc_th, (P, NTH)).copy()

    nc2 = _get_k2()
    in2 = [{"unc": o["unc"], "Aw": o["Aw"], "Pw": o["Pw"],
            "Bw": o["Bw"], "Qw": o["Qw"], "th": th_b} for o in outs1]
    tr2 = {"trace": True, "tmpdir": "/tmp/trace_k2"} if _trace else {}
    r2 = _run(nc2, in2, **tr2)
    outs2 = r2.results

    S_le = np.zeros((NTH, 4), dtype=np.float64)  # A, P, B, Q
    for o in outs2:
        S_le += o["accv"].astype(np.float64).sum(axis=0).reshape(NTH, 4)

    n_ac = S_le[:, 0] - S_le[:, 1]
    n_au = SP - S_le[:, 1]
    n_ic = S_le[:, 2] - S_le[:, 3]
    n_iu = SQ - S_le[:, 3]
    avu = (n_ac + n_iu) / (n_ac + n_au + n_ic + n_iu + EPS)
    dx = np.diff(th01.astype(np.float64))
    auc_avu = float(np.sum(0.5 * (avu[1:] + avu[:-1]) * dx))
    avu_loss = -BETA * np.log(auc_avu + EPS) + focal_loss

    if _results is not None:
        _results.update(r1=r1, r2=r2, umin=umin, umax=umax, n=np.stack(
            [n_ac, n_au, n_ic, n_iu]), avu=avu, auc=auc_avu)
    return (np.float32(avu_loss), np.float32(CE_loss))
